# revision 27
# baseline (speedup 1.0000x reference)
"""Dense multi-head attention (B=4, H=16, N=2048, D=64) on 8 trn2 NeuronCores.

Sharding: batch*head parallel - 64 (b,h) pairs, 8 per core. Each core runs a
fused flash-style attention over its heads.

Final design (289us HW; the v2 baseline was 490us):
  - All matmuls in ONE 128x128 array config: Q^T/K^T are zero-padded to
    128 partitions. v2 mixed 64x128 S tiles with 128x128 O tiles; the
    per-iteration array reconfig kept the PE HAM clock-gate at K=4/8
    (1.2 GHz, 609ns/MM) for most of the kernel. Uniform config keeps it
    warm (2.4 GHz, 379ns/MM, 216ns b2b) — the single biggest win.
  - exp split across engines at the PSUM bank boundary so the probs path
    never binds: ScalarE exact exp on cols [0,512) of its own PSUM tile
    s_psA, DVE Schraudolph bit-trick exp (int16 bitcast of a bf16 tile —
    bitcast keeps Tile dep tracking) on [512,1024) from s_psB. Separate
    tiles/banks mean each S matmul's bank-reuse dep is single-engine.
    Schraudolph on 50% of columns costs ~6e-3 extra rel err
    (1.15e-2 total vs 2e-2 budget).
  - Software pipelining: O matmuls lag their probs producers by TWO
    k-blocks so the in-order PE queue never waits on exp.
  - denominator via 32 replicated ones-columns in va (o_ps rows 64:95);
    ONE xbar DMA transpose [96,1024]->[128,8,96] (SP queue) moves outputs
    AND denominators; DVE reciprocal runs on 128 lanes ([128,8], 200ns)
    instead of v2's 1-lane [1,1024] disaster (6.5us); normalize muls on
    the otherwise-idle GpSimd (on DVE for the final head = 18us less
    serial kernel tail).
  - epilogue emitted as delayed closures flushed at mp slots 3,4,5 /
    13,14,15 of the NEXT q-chunk: the obf copy is chunked so it displaces
    ScalarE exps by <700ns, and the DVE recip is emitted ~8us after the
    transpose whose semaphore it waits on — any DVE-queue op parking on
    a late semaphore stalls the PE via s_psB bank reuse (and each such
    stall re-triggers a cold HAM window, cascading into the next chunk).
No max-subtraction pass: scores/8 ~ N(0,1); exp stays well inside f32/bf16
range, matching jax.nn.softmax to bf16 precision.
"""

import os
import sys

import numpy as np

for _p in ("/opt/trn_rl_repo", "/root/.axon_site/_ro/trn_rl_repo"):
    if os.path.isdir(_p) and _p not in sys.path:
        sys.path.insert(0, _p)

import ml_dtypes

B, H, N, D = 4, 16, 2048, 64
NCORES = 8
HPC = B * H // NCORES  # heads (b,h pairs) per core = 8
BF16 = ml_dtypes.bfloat16

# Schraudolph exp: bf16 bits ~= rint(A_SCH * s + B_SCH) for exp(s/8)
A_SCH = 0.125 * float(np.log2(np.e)) * 128.0  # 23.083120
B_SCH = 16256.0 - 7.4  # 127*128 + fitted log-centering correction
SPLIT = 512  # cols [0,SPLIT) exact exp on ScalarE; rest Schraudolph on DVE
# 512 = PSUM bank boundary: ScalarE and DVE each own a whole bank, so the
# next S matmul pair's bank-reuse deps are single-engine (no cross waits).
VPAD = 32  # ones-columns appended to V (denominator rows 64:96 of o_ps)

_CACHE = {}


def _build_nc(split=SPLIT):
    import concourse.bass as bass
    import concourse.mybir as mybir
    import concourse.tile as tile
    from concourse import bacc

    bf16 = mybir.dt.bfloat16
    f32 = mybir.dt.float32
    i16 = mybir.dt.int16

    QC = 1024         # q chunk (PSUM: [128, QC] f32 = 2 banks)
    NQC = N // QC     # 2 q-chunks per head
    MP = N // 128     # 16 k-pair blocks (2 x 64) per head
    QB = QC // 128    # 8 128-row q blocks per chunk

    DV = D + VPAD  # 96: V cols 0:64, ones cols 64:96 (denominator rows)

    nc = bacc.Bacc(
        "TRN2", target_bir_lowering=False, debug=False, num_devices=NCORES
    )
    # q/k padded to 128 partitions (rows 64:128 zero) so every matmul runs
    # in the SAME 128x128 array config: mixing 64x128 S tiles with 128x128
    # O tiles forces a PE array drain/reconfig between them every
    # iteration, and empirically keeps the HAM clock-gate at K=4/8
    # (1.2 GHz) for the whole kernel.
    qt = nc.declare_dram_parameter("qt", [HPC, 128, N], bf16, isOutput=False)
    kt = nc.declare_dram_parameter("kt", [HPC, 128, N], bf16, isOutput=False)
    va = nc.declare_dram_parameter("va", [HPC, N, DV], bf16, isOutput=False)
    out = nc.declare_dram_parameter("out", [HPC, N, D], bf16, isOutput=True)

    with tile.TileContext(nc) as tc:
        with (
            tc.sbuf_pool(name="inp", bufs=2) as inp,
            tc.sbuf_pool(name="probs", bufs=6) as probs,
            tc.sbuf_pool(name="epil", bufs=2) as epil,
            tc.psum_pool(name="spsumA", bufs=2) as spsumA,
            tc.psum_pool(name="spsumB", bufs=2) as spsumB,
            tc.psum_pool(name="opsum", bufs=2) as opsum,
        ):
            epi_pend = []  # delayed epilogue emission (keeps DVE queue clear)

            def emit_head(h):
                # head 0: halved DMAs so the first S matmuls start ~5us
                # earlier. Later heads: 3 whole DMAs (less SP-queue
                # occupancy ahead of the pending epilogue transposes).
                kt_t = inp.tile([128, N], bf16, tag="kt", name="kt_t")
                qt_t = inp.tile([128, N], bf16, tag="qt", name="qt_t")
                va_t = inp.tile([128, MP, DV], bf16, tag="va", name="va_t")
                va_r = va[h].rearrange("(m p) d -> p m d", p=128)
                nc.sync.dma_start(out=kt_t[:, 0 : N // 2], in_=kt[h][:, 0 : N // 2])
                nc.sync.dma_start(out=qt_t[:, 0 : N // 2], in_=qt[h][:, 0 : N // 2])
                nc.sync.dma_start(out=va_t[:, 0 : MP // 2], in_=va_r[:, 0 : MP // 2])
                nc.sync.dma_start(out=kt_t[:, N // 2 : N], in_=kt[h][:, N // 2 : N])
                nc.sync.dma_start(out=qt_t[:, N // 2 : N], in_=qt[h][:, N // 2 : N])
                nc.sync.dma_start(out=va_t[:, MP // 2 : MP], in_=va_r[:, MP // 2 : MP])
                out_t = epil.tile([128, N // 128, D], bf16, tag="out", name="out_t")

                for qc in range(NQC):
                    o_ps = opsum.tile([DV, QC], f32, tag="o", name="o_ps")
                    pend = []

                    def emit_o(mp, p_ab, o_ps=o_ps, va_t=va_t):
                        for u, p_u in enumerate(p_ab):
                            nc.tensor.matmul(
                                o_ps[:, u * 512 : (u + 1) * 512],
                                va_t[:, mp, :],
                                p_u,
                                start=(mp == 0),
                                stop=(mp == MP - 1),
                            )

                    for mp in range(MP):
                        # 2 S matmuls: [128,128] stationary (rows 64:128
                        # zero) -> [128,512] out; same config as O mms.
                        # Separate single-bank PSUM tiles: ScalarE exp owns
                        # bank A, DVE Schraudolph owns bank B, so each next
                        # S matmul waits on exactly one consumer engine.
                        st = kt_t[:, mp * 128 : (mp + 1) * 128]
                        s_psA = spsumA.tile([128, split], f32, tag="sA", name="s_psA")
                        nc.tensor.matmul(
                            s_psA,
                            st,
                            qt_t[:, qc * QC : qc * QC + split],
                            start=True,
                            stop=True,
                        )
                        s_psB = spsumB.tile([128, QC - split], f32, tag="sB", name="s_psB")
                        nc.tensor.matmul(
                            s_psB,
                            st,
                            qt_t[:, qc * QC + split : qc * QC + QC],
                            start=True,
                            stop=True,
                        )
                        # separate p_a/p_b tiles: exp and Schraudolph have
                        # no common operand, so neither engine waits on the
                        # other (split == the O-matmul 512-col chunking).
                        p_a = probs.tile([128, split], bf16, tag="pa", name="p_a")
                        nc.scalar.activation(
                            p_a,
                            s_psA,
                            mybir.ActivationFunctionType.Exp,
                            scale=0.125,
                        )
                        # int16 bitcast view: Schraudolph bits land as
                        # bf16; bitcast keeps Tile dep tracking (a raw
                        # SBTensorHandle alias would not).
                        p_b = probs.tile([128, QC - split], bf16, tag="pb", name="p_b")
                        nc.vector.tensor_scalar(
                            p_b.bitcast(i16),
                            s_psB,
                            A_SCH,
                            B_SCH,
                            mybir.AluOpType.mult,
                            mybir.AluOpType.add,
                        )
                        pend.append((mp, (p_a, p_b)))
                        if len(pend) > 2:
                            omp, op = pend.pop(0)
                            emit_o(omp, op)
                        if epi_pend and mp in (3, 4, 5, 13, 14, 15):
                            epi_pend.pop(0)()
                    for omp, op in pend:
                        emit_o(omp, op)

                    # epilogue as small delayed stages, each flushed at a
                    # different mp slot: the obf copy is chunked so it
                    # never displaces ScalarE exps by more than ~700ns, the
                    # denominator transpose goes first (small) so the DVE
                    # recip emitted 6 slots later never parks at the DVE
                    # queue head waiting for transpose semaphores.
                    state = {}

                    is_tail = h == HPC - 1

                    def epi_c0(o_ps=o_ps, state=state):
                        obf = epil.tile([DV, QC], bf16, tag="obf", name="obf")
                        nc.scalar.copy(obf[:, 0 : QC // 2], o_ps[:, 0 : QC // 2])
                        state["obf"] = obf

                    def epi_c1(o_ps=o_ps, state=state):
                        obf = state["obf"]
                        nc.scalar.copy(obf[:, QC // 2 : QC], o_ps[:, QC // 2 : QC])

                    def epi_tpose(state=state):
                        o_T = epil.tile([128, QB, DV], bf16, tag="oT", name="o_T")
                        nc.sync.dma_start_transpose(o_T, state["obf"])
                        state["o_T"] = o_T

                    def epi_rec(state=state):
                        recT = epil.tile([128, QB], f32, tag="recT", name="recT")
                        nc.vector.reciprocal(recT, state["o_T"][:, :, D : D + 1])
                        state["recT"] = recT

                    def epi_mul(qc=qc, out_t=out_t, state=state, is_tail=is_tail):
                        # normalize on (otherwise idle) GpSimd, off the
                        # PE/ScalarE/DVE critical engines. The final
                        # head-qc has nothing left to overlap, so run its
                        # muls on the ~13x faster DVE to cut the kernel
                        # tail (GpSimd: 1.15us/instr, 18us serial tail).
                        eng = nc.vector if is_tail else nc.gpsimd
                        for j in range(QB):
                            eng.tensor_scalar_mul(
                                out_t[:, qc * QB + j, :],
                                state["o_T"][:, j, 0:D],
                                state["recT"][:, j : j + 1],
                            )

                    epi_pend.extend([epi_c0, epi_c1, epi_tpose, epi_rec, epi_mul])

                def out_dma(h=h, out_t=out_t):
                    nc.sync.dma_start(
                        out=out[h].rearrange("(m p) d -> p m d", p=128), in_=out_t
                    )

                epi_pend.append(out_dma)

            for h in range(HPC):
                emit_head(h)
            while epi_pend:
                epi_pend.pop(0)()
    nc.compile()
    return nc


def _get_nc():
    if "nc" not in _CACHE:
        _CACHE["nc"] = _build_nc()
    return _CACHE["nc"]


def _prep_shards(q, k, v):
    """Host-side: split heads, cast bf16 (round-to-nearest-even, matching the
    reference's astype), transpose Q/K to [d, n] padded to 128 rows with
    zeros (uniform 128x128 matmul config), append ones columns to V."""
    q4t = q.reshape(B, N, H, D).transpose(0, 2, 3, 1).reshape(B * H, D, N)
    k4t = k.reshape(B, N, H, D).transpose(0, 2, 3, 1).reshape(B * H, D, N)
    q4 = np.zeros((B * H, 128, N), dtype=BF16)
    q4[:, :D] = q4t.astype(BF16)
    k4 = np.zeros((B * H, 128, N), dtype=BF16)
    k4[:, :D] = k4t.astype(BF16)
    v4 = np.ascontiguousarray(
        v.reshape(B, N, H, D).transpose(0, 2, 1, 3).reshape(B * H, N, D)
    ).astype(BF16)
    ones = np.ones((B * H, N, VPAD), dtype=BF16)
    va = np.concatenate([v4, ones], axis=2)

    in_maps = []
    for c in range(NCORES):
        sl = slice(c * HPC, (c + 1) * HPC)
        in_maps.append(
            {
                "qt": np.ascontiguousarray(q4[sl]),
                "kt": np.ascontiguousarray(k4[sl]),
                "va": np.ascontiguousarray(va[sl]),
            }
        )
    return in_maps


def _make_runner():
    """Persistent jitted SPMD executor (mirrors bass2jax.run_bass_via_pjrt but
    reusable across calls, no donation so device inputs can be reused)."""
    import jax
    import numpy as _np
    from jax.sharding import Mesh, PartitionSpec
    from concourse import bass2jax, mybir

    try:
        from jax.experimental.shard_map import shard_map
    except ImportError:
        shard_map = jax.shard_map

    bass2jax.install_neuronx_cc_hook()
    nc = _get_nc()

    partition_name = (
        nc.partition_id_tensor.name if nc.partition_id_tensor is not None else None
    )
    in_names, out_names, out_avals, zero_outs = [], [], [], []
    for alloc in nc.m.functions[0].allocations:
        if not isinstance(alloc, mybir.MemoryLocationSet):
            continue
        name = alloc.memorylocations[0].name
        if alloc.kind == "ExternalInput":
            if name != partition_name:
                in_names.append(name)
        elif alloc.kind == "ExternalOutput":
            out_names.append(name)
            shape = tuple(alloc.tensor_shape)
            dtype = mybir.dt.np(alloc.dtype)
            out_avals.append(jax.core.ShapedArray(shape, dtype))
            zero_outs.append(_np.zeros(shape, dtype))
    n_params = len(in_names)

    all_in_names = in_names + out_names
    if partition_name is not None:
        all_in_names = all_in_names + [partition_name]

    def _body(*args):
        operands = list(args)
        if partition_name is not None:
            operands.append(bass2jax.partition_id_tensor())
        outs = bass2jax._bass_exec_p.bind(
            *operands,
            out_avals=tuple(out_avals),
            in_names=tuple(all_in_names),
            out_names=tuple(out_names),
            lowering_input_output_aliases=(),
            sim_require_finite=True,
            sim_require_nnan=True,
            nc=nc,
        )
        return tuple(outs)

    devices = jax.devices()[:NCORES]
    mesh = Mesh(np.asarray(devices), ("core",))
    in_specs = (PartitionSpec("core"),) * (n_params + len(out_names))
    out_specs = (PartitionSpec("core"),) * len(out_names)
    sharded = jax.jit(
        shard_map(
            _body, mesh=mesh, in_specs=in_specs, out_specs=out_specs, check_rep=False
        ),
        keep_unused=True,
    )

    def run(in_maps):
        concat_in = [
            np.concatenate([in_maps[c][nm] for c in range(NCORES)], axis=0)
            for nm in in_names
        ]
        concat_zeros = [
            np.zeros((NCORES * z.shape[0], *z.shape[1:]), z.dtype) for z in zero_outs
        ]
        out_arrs = sharded(*concat_in, *concat_zeros)
        return [
            {
                nm: np.asarray(out_arrs[i]).reshape(NCORES, *out_avals[i].shape)[c]
                for i, nm in enumerate(out_names)
            }
            for c in range(NCORES)
        ]

    def put(in_maps):
        import jax as _jax
        from jax.sharding import NamedSharding

        sh = NamedSharding(mesh, PartitionSpec("core"))
        concat_in = [
            np.concatenate([in_maps[c][nm] for c in range(NCORES)], axis=0)
            for nm in in_names
        ]
        concat_zeros = [
            np.zeros((NCORES * z.shape[0], *z.shape[1:]), z.dtype) for z in zero_outs
        ]
        return [_jax.device_put(x, sh) for x in concat_in + concat_zeros]

    return {"run": run, "put": put, "sharded": sharded}


def _get_runner():
    if "runner" not in _CACHE:
        _CACHE["runner"] = _make_runner()
    return _CACHE["runner"]


def timed_run(in_maps, iters=10):
    """Return (best_wall_seconds_per_call, results). Device-resident inputs."""
    import time

    import jax

    r = _get_runner()
    args = r["put"](in_maps)
    out = r["sharded"](*args)
    jax.block_until_ready(out)
    best = float("inf")
    for _ in range(iters):
        t0 = time.perf_counter()
        out = r["sharded"](*args)
        jax.block_until_ready(out)
        best = min(best, time.perf_counter() - t0)
    return best, out


def kernel(q, k, v):
    q = np.asarray(q, dtype=np.float32)
    k = np.asarray(k, dtype=np.float32)
    v = np.asarray(v, dtype=np.float32)
    in_maps = _prep_shards(q, k, v)

    res = _get_runner()["run"](in_maps)

    outs = [np.asarray(res[c]["out"]) for c in range(NCORES)]
    out_all = np.concatenate(outs, axis=0)  # [B*H, N, D] bf16
    full = (
        out_all.reshape(B, H, N, D).transpose(0, 2, 1, 3).reshape(B, N, H * D)
    )
    return np.ascontiguousarray(full)

